# revision 1
# baseline (speedup 1.0000x reference)
"""GroupedQueryAttention (head-axis-contracting variant) on 8 TRN2 NeuronCores.

Reference computation (B=2, S=2048, E=4096, D=128, H=32, Hkv=8, scale=4):
    q = einsum('bse,edh->bsdh', x, Wq) + bq          [B,S,D,H]
    k,v likewise with Hkv heads, then repeated 4x along h
    scores = einsum('bsdh,bseh->bsde', q, k) / sqrt(D)   (contracts the HEAD axis)
    out = softmax(scores, -1) @ v  -> reshape [B,S,E]

Because the head axis is contracted, q only enters through group-sums over the
4 q-heads sharing each kv head, and out's 4 head-columns per group are equal.
Per token the kernel computes:
    scoresT[e,d] = sum_g ksum... k[g,e] * qsum[g,d]        (K=8 matmul)
    E = exp(scoresT)                                        (no max-subtract;
                                                             |scores| < ~8)
    U[g|s, d] = [v | ones]^T @ E                            (K=128 matmul)
    out[t, d*32 + 4g+j] = U[g,d] / U[8,d]

Sharding: pure data-parallel over the 4096 tokens, 512 per core; weights
replicated. Host pre-work is layout/precision only (group-sum of Wq, bf16
casts, transposes); all FLOPs of the math above run on device.
"""

import os
import numpy as np
import ml_dtypes

_PHASES = os.environ.get("K_PHASES", "all")  # all | proj | nofin

import concourse.bass as bass
import concourse.mybir as mybir
import concourse.tile as tile
from concourse.vector_clock import ScopedClock

BF = ml_dtypes.bfloat16
F32 = mybir.dt.float32
BF16 = mybir.dt.bfloat16
AF = mybir.ActivationFunctionType

E, D, H, G, SC = 4096, 128, 32, 8, 4
B, S = 2, 2048
T = B * S
NCORES = 8
TPC = T // NCORES          # 512 tokens per core
KT = E // 128              # 32 contraction tiles
RCH = 32                   # stage-C / output token chunk
NCH = TPC // RCH           # 16 chunks


_MAXW = 1  # max sync-waits left on any one instruction


class _SplitDrainTileContext(tile.TileContext):
    """Workaround: this walrus build caps sync-wait commands per instruction.
    Spill excess waits onto same-engine nops inserted just before the
    instruction (same-engine stream order makes that equivalent), and do the
    same for the kernel-tail Drain."""

    def _add_instruction(self, inst):
        si = inst.sync_info
        if si is not None and si.on_wait and len(si.on_wait) > _MAXW:
            waits = list(si.on_wait)
            si.on_wait = waits[:_MAXW]
            for i in range(_MAXW, len(waits), _MAXW):
                nop = mybir.InstNoOp(
                    name=self.nc.get_next_instruction_name(),
                    engine=inst.engine, ins=[], outs=[],
                )
                nop.sync_info = mybir.SyncInfo(
                    on_wait=waits[i : i + _MAXW], on_update=[]
                )
                super()._add_instruction(nop)
        super()._add_instruction(inst)

    def _drain_and_barrier(self, tick_clock, wait_clock):
        nc = self.nc
        carrier = nc.sync.nop(nofuse=True).ins
        wait_clock.add_sem_waits(carrier, ScopedClock({None: tick_clock.global_clock}))
        waits = list(carrier.sync_info.on_wait) if carrier.sync_info else []
        if len(waits) > 1:
            carrier.sync_info.on_wait = waits[:1]
            for w in waits[1:]:
                extra = nc.sync.nop(nofuse=True).ins
                extra.sync_info = mybir.SyncInfo(on_wait=[w], on_update=[])
        nc.sync.drain()
        nc.all_engine_barrier()
        assert self.sems is not None
        popped = nc._tile_sem_poison_stack.pop()
        assert popped is self._sem_poison
        nc.clear_and_free_semaphores(list(self.sems.allocated().values()))
        nc.all_engine_barrier()


def _emit_body(nc, params, rep):
    """Emit one full forward pass. `params` maps name -> DRAM handle."""
    xw, wq, wk, wv, bq2, bk2, bv2, out_ext = (
        params["xw"], params["wq"], params["wk"], params["wv"],
        params["bq2"], params["bk2"], params["bv2"], params["out"],
    )
    tc = params["_tc"]
    with (
        tc.tile_pool(name=f"sbA{rep}", bufs=1) as sbA,
        tc.tile_pool(name=f"wp{rep}", bufs=2) as wpool,
        tc.tile_pool(name=f"pp{rep}", bufs=2, space="PSUM") as ppool,
        tc.tile_pool(name=f"gp{rep}", bufs=2) as gpool,
        tc.tile_pool(name=f"sp{rep}", bufs=2, space="PSUM") as spool,
        tc.tile_pool(name=f"up{rep}", bufs=2, space="PSUM") as upool,
        tc.tile_pool(name=f"ep{rep}", bufs=3) as epool,
        tc.tile_pool(name=f"ub{rep}", bufs=2) as ubpool,
        tc.tile_pool(name=f"fin{rep}", bufs=2) as fpool,
        tc.tile_pool(name=f"dr{rep}", bufs=1, space="DRAM") as dpool,
    ):
        # ---- resident inputs
        xsb = sbA.tile([128, KT * TPC], BF16, tag="xsb")       # [e_lo, (k, t)]
        nc.sync.dma_start(out=xsb[:], in_=xw[:])
        qsb = sbA.tile([128, G * TPC], BF16, tag="qsb")        # [d, (g, t)]
        ksb = sbA.tile([128, G * TPC], BF16, tag="ksb")
        vaug = sbA.tile([128, (G + 1) * TPC], BF16, tag="vaug")  # [dv,(g,t)]+ones
        nc.vector.memset(vaug[:, G * TPC :], 1.0)
        bq_sb = sbA.tile([128, G], F32, tag="bq_sb")
        bk_sb = sbA.tile([128, G], F32, tag="bk_sb")
        bv_sb = sbA.tile([128, G], F32, tag="bv_sb")
        nc.sync.dma_start(out=bq_sb[:], in_=bq2[:])
        nc.sync.dma_start(out=bk_sb[:], in_=bk2[:])
        nc.sync.dma_start(out=bv_sb[:], in_=bv2[:])

        # ---- projections: dest[:, g*TPC:(g+1)*TPC] = W_g^T @ xT (+ bias)
        for wext, dest, bias in ((wq, qsb, bq_sb), (wk, ksb, bk_sb), (wv, vaug, bv_sb)):
            for g in range(G):
                wtile = wpool.tile([128, KT * 128], BF16, tag="wtile")
                nc.sync.dma_start(out=wtile[:], in_=wext[g])
                psum = ppool.tile([128, TPC], F32, tag="psum")
                for k in range(KT):
                    nc.tensor.matmul(
                        psum[:],
                        wtile[:, k * 128 : (k + 1) * 128],
                        xsb[:, k * TPC : (k + 1) * TPC],
                        start=(k == 0),
                        stop=(k == KT - 1),
                    )
                nc.scalar.activation(
                    dest[:, g * TPC : (g + 1) * TPC], psum[:], AF.Identity,
                    bias=bias[:, g : g + 1],
                )

        # ---- bounce q/k through DRAM so stage-C gathers are 1 DMA each
        # (d-major layout: store order (d, g, t) matches qsb's linear order)
        q_dr = dpool.tile([D, G, TPC], BF16, tag="q_dr")
        k_dr = dpool.tile([D, G, TPC], BF16, tag="k_dr")
        a_dr = dpool.tile([NCH, D, RCH, G], F32, tag="a_dr")
        nc.sync.dma_start(out=q_dr[:], in_=qsb[:])
        nc.sync.dma_start(out=k_dr[:], in_=ksb[:])

        # ---- stage C, chunked over tokens
        for c in range(NCH if _PHASES != "proj" else 0):
            t0 = c * RCH
            # gather qg/kg [8 g, (d, t)] from DRAM (permuted DRAM-side AP)
            qg = gpool.tile([G, D * RCH], BF16, tag="qg")
            kg = gpool.tile([G, D * RCH], BF16, tag="kg")
            nc.sync.dma_start(
                out=qg[:], in_=q_dr[:, :, t0 : t0 + RCH].transpose([1, 0, 2])
            )
            nc.sync.dma_start(
                out=kg[:], in_=k_dr[:, :, t0 : t0 + RCH].transpose([1, 0, 2])
            )
            qgv = qg[:].rearrange("g (d t) -> g t d", t=RCH)
            kgv = kg[:].rearrange("g (d t) -> g t d", t=RCH)
            vv = vaug[:].rearrange("p (n t) -> p t n", t=TPC)
            # U' [128 d, 16-per-token (8 v-cols | s | pad)] packed chunk-wide
            ups2 = upool.tile([128, RCH * 16], F32, tag="ups2")
            for quad in range(RCH // 4):
                ps4 = spool.tile([128, 512], F32, tag="ps4")
                for i in range(4):
                    tl = quad * 4 + i
                    nc.tensor.matmul(
                        ps4[:, i * D : (i + 1) * D],
                        kgv[:, tl, :], qgv[:, tl, :],
                        start=True, stop=True,
                    )
                e4 = epool.tile([128, 512], BF16, tag="e4")
                nc.scalar.activation(e4[:], ps4[:], AF.Exp)
                for i in range(4):
                    tl = quad * 4 + i
                    nc.tensor.matmul(
                        ups2[:, tl * 16 : tl * 16 + 9],
                        e4[:, i * D : (i + 1) * D], vv[:, t0 + tl, :],
                        start=True, stop=True,
                    )

            # ---- finalize: one evacuation, normalize in d-major, transpose
            # via DRAM, duplicate 4x on the way out
            if _PHASES == "nofin":
                continue
            usb2 = ubpool.tile([128, RCH * 9], F32, tag="usb2")
            nc.vector.tensor_copy(
                usb2[:].rearrange("d (t s) -> d t s", s=9),
                ups2[:].rearrange("d (t s) -> d t s", s=16)[:, :, 0:9],
            )
            rtd = fpool.tile([128, RCH], F32, tag="rtd")
            uview = usb2[:].rearrange("d (t s) -> d t s", s=9)
            nc.vector.reciprocal(rtd[:], uview[:, :, 8])
            attn_n = fpool.tile([128, RCH * G], F32, tag="attn_n")
            nc.vector.tensor_tensor(
                attn_n[:].rearrange("d (t g) -> d t g", g=G),
                uview[:, :, 0:G],
                rtd[:].unsqueeze(2).broadcast_to([128, RCH, G]),
                op=mybir.AluOpType.mult,
            )
            nc.sync.dma_start(out=a_dr[c], in_=attn_n[:])
            atok = fpool.tile([RCH, D * G], F32, tag="atok")   # [t, (d, g)]
            nc.sync.dma_start(out=atok[:], in_=a_dr[c].transpose([1, 0, 2]))
            om = fpool.tile([RCH, D * H], F32, tag="om")
            nc.vector.tensor_copy(
                om[:].rearrange("t (d g j) -> t d g j", g=G, j=SC),
                atok[:].rearrange("t (d g) -> t d g", g=G)
                .unsqueeze(3).broadcast_to([RCH, D, G, SC]),
            )
            nc.sync.dma_start(out=out_ext[t0 : t0 + RCH, :], in_=om[:])


def build_program(reps=1):
    """Build the SPMD single-core program; same NEFF runs on all 8 cores."""
    nc = bass.Bass("TRN2", target_bir_lowering=False, debug=False,
                   num_devices=NCORES)
    params = {
        "xw": nc.declare_dram_parameter("xw", [128, KT, TPC], BF16, isOutput=False),
        "wq": nc.declare_dram_parameter("wq", [G, 128, KT, 128], BF16, isOutput=False),
        "wk": nc.declare_dram_parameter("wk", [G, 128, KT, 128], BF16, isOutput=False),
        "wv": nc.declare_dram_parameter("wv", [G, 128, KT, 128], BF16, isOutput=False),
        "bq2": nc.declare_dram_parameter("bq2", [128, G], F32, isOutput=False),
        "bk2": nc.declare_dram_parameter("bk2", [128, G], F32, isOutput=False),
        "bv2": nc.declare_dram_parameter("bv2", [128, G], F32, isOutput=False),
        "out": nc.declare_dram_parameter("out", [TPC, D * H], F32, isOutput=True),
    }
    with _SplitDrainTileContext(nc) as tc:
        params["_tc"] = tc
        for rep in range(reps):
            _emit_body(nc, params, rep)
    del params["_tc"]
    return nc


def prepare_inputs(x, Wq, bq, Wk, bk, Wv, bv):
    """Host-side sharding + layout/precision transforms -> per-core in_maps."""
    x = np.asarray(x, np.float32)
    scale = np.float32(1.0 / np.sqrt(D))

    def wmat(W, do_sum):
        W = np.asarray(W, np.float32)
        if do_sum:
            W = W.reshape(E, D, G, SC).sum(axis=3) * scale
        # [E, D, G] -> [E, g*128+d] -> [g, p, k, c] device tile layout
        m = W.transpose(0, 2, 1).reshape(E, G * D)
        return np.ascontiguousarray(
            m.reshape(KT, 128, G, D).transpose(2, 1, 0, 3)
        ).astype(BF)

    wq_h = wmat(Wq, True)
    wk_h = wmat(Wk, False)
    wv_h = wmat(Wv, False)
    bq_h = (np.asarray(bq, np.float32).reshape(D, G, SC).sum(axis=2) * scale)
    bk_h = np.ascontiguousarray(np.asarray(bk, np.float32))
    bv_h = np.ascontiguousarray(np.asarray(bv, np.float32))

    x_flat = x.reshape(T, E)
    in_maps = []
    for i in range(NCORES):
        xT = x_flat[i * TPC : (i + 1) * TPC].T          # [E, TPC]
        xw = xT.reshape(KT, 128, TPC).transpose(1, 0, 2).astype(BF)
        in_maps.append({
            "xw": np.ascontiguousarray(xw),
            "wq": wq_h, "wk": wk_h, "wv": wv_h,
            "bq2": bq_h, "bk2": bk_h, "bv2": bv_h,
        })
    return in_maps


def prepare_inputs_single(x, Wq, bq, Wk, bk, Wv, bv):
    """One-core variant for simulation: x must hold exactly TPC tokens."""
    x = np.asarray(x, np.float32).reshape(TPC, E)
    maps = prepare_inputs(
        np.broadcast_to(x.reshape(1, TPC, E), (NCORES, TPC, E)).reshape(B, S, E),
        Wq, bq, Wk, bk, Wv, bv,
    )
    return maps[0]


_CACHED = {}


def kernel(x, Wq, bq, Wk, bk, Wv, bv):
    from concourse.bass_utils import run_bass_kernel_spmd

    if "nc" not in _CACHED:
        _CACHED["nc"] = build_program(reps=1)
    nc = _CACHED["nc"]
    in_maps = prepare_inputs(x, Wq, bq, Wk, bk, Wv, bv)
    res = run_bass_kernel_spmd(nc, in_maps, list(range(NCORES)), trace=False)
    out = np.concatenate([res.results[i]["out"] for i in range(NCORES)], axis=0)
    return out.reshape(B, S, E).astype(np.float32)



# revision 19
# speedup vs baseline: 1.7840x; 1.7840x over previous
"""GroupedQueryAttention (head-axis-contracting variant) on 8 TRN2 NeuronCores.

Reference computation (B=2, S=2048, E=4096, D=128, H=32, Hkv=8, scale=4):
    q = einsum('bse,edh->bsdh', x, Wq) + bq          [B,S,D,H]
    k,v likewise with Hkv heads, then repeated 4x along h
    scores = einsum('bsdh,bseh->bsde', q, k) / sqrt(D)   (contracts the HEAD axis)
    out = softmax(scores, -1) @ v  -> reshape [B,S,E]

Because the head axis is contracted, q only enters through group-sums over the
4 q-heads sharing each kv head, and out's 4 head-columns per group are equal.
Per token the kernel computes:
    scoresT[e,d] = sum_g k[g,e] * qsum[g,d]                 (K=8 matmul)
    E = exp(scoresT)                                        (|scores| < ~8)
    U[g|s, d] = [v | ones]^T @ E                            (K=128 matmul)
    out[d, t, g] = U[g,d] / U[8,d]   (head 4x duplication on host: layout only)

Projections run in fp8e4 DoubleRow mode (2 contraction rows/partition): both
x and W are split hi+lo in e4m3, and 3 of the 4 cross products are computed
(hi*hi, lo*hi, hi*lo), restoring ~bf16 accuracy at ~0.75x of the bf16 matmul
cycles. W is prescaled by 256 so its (tiny) values sit in e4m3's normal range;
the psum evacuation activation undoes it via scale=1/256.

Sharding: pure data-parallel over the 4096 tokens, 512 per core; weights
replicated. Host pre-work is layout/precision only (group-sum of Wq, fp8
hi/lo splits, transposes, final head duplication); all FLOPs run on device.
"""

import numpy as np
import ml_dtypes

import concourse.bass as bass
import concourse.mybir as mybir
import concourse.tile as tile
from concourse.vector_clock import ScopedClock

BF = ml_dtypes.bfloat16
F8 = ml_dtypes.float8_e4m3
F32 = mybir.dt.float32
BF16 = mybir.dt.bfloat16
FP8 = mybir.dt.float8e4
AF = mybir.ActivationFunctionType
DR = mybir.MatmulPerfMode.DoubleRow

E, D, H, G, SC = 4096, 128, 32, 8, 4
B, S = 2, 2048
T = B * S
NCORES = 8
TPC = T // NCORES          # 512 tokens per core
KT = E // 128              # 32 contraction k-subtiles
CB = 32                    # stage-C block (gather/U/finalize unit)
NCB = TPC // CB            # 16 blocks
SBT = 8                    # tokens per exp sub-batch (1024 psum cols)
NSB = CB // SBT            # 4 sub-batches per block
NEB = 13                   # E-tile pool depth (SBUF-limited)
DEFER = 12                 # blocks whose scores interleave with V projections
VCHUNK = 8                 # projection matmuls interleaved per score sub-batch
WPS = 256.0                # fp8 weight prescale

_MAXW = 1  # max sync-waits left on any one instruction


class _SplitDrainTileContext(tile.TileContext):
    """Workaround: this walrus build caps sync-wait commands per instruction.
    Spill excess waits onto same-engine nops inserted just before the
    instruction (same-engine stream order makes that equivalent), and do the
    same for the kernel-tail Drain."""

    def _add_instruction(self, inst):
        si = inst.sync_info
        if si is not None and si.on_wait and len(si.on_wait) > _MAXW:
            waits = list(si.on_wait)
            si.on_wait = waits[:_MAXW]
            for i in range(_MAXW, len(waits), _MAXW):
                nop = mybir.InstNoOp(
                    name=self.nc.get_next_instruction_name(),
                    engine=inst.engine, ins=[], outs=[],
                )
                nop.sync_info = mybir.SyncInfo(
                    on_wait=waits[i : i + _MAXW], on_update=[]
                )
                super()._add_instruction(nop)
        super()._add_instruction(inst)

    def _drain_and_barrier(self, tick_clock, wait_clock):
        nc = self.nc
        carrier = nc.sync.nop(nofuse=True).ins
        wait_clock.add_sem_waits(carrier, ScopedClock({None: tick_clock.global_clock}))
        waits = list(carrier.sync_info.on_wait) if carrier.sync_info else []
        if len(waits) > 1:
            carrier.sync_info.on_wait = waits[:1]
            for w in waits[1:]:
                extra = nc.sync.nop(nofuse=True).ins
                extra.sync_info = mybir.SyncInfo(on_wait=[w], on_update=[])
        nc.sync.drain()
        nc.all_engine_barrier()
        assert self.sems is not None
        popped = nc._tile_sem_poison_stack.pop()
        assert popped is self._sem_poison
        nc.clear_and_free_semaphores(list(self.sems.allocated().values()))
        nc.all_engine_barrier()


def _emit_body(nc, params, rep):
    """Emit one full forward pass. `params` maps name -> DRAM handle."""
    xhi_ext, xlo_ext, wq, wk, wv, ball, out_ext = (
        params["xhi"], params["xlo"], params["wq"], params["wk"], params["wv"],
        params["ball"], params["out"],
    )
    tc = params["_tc"]
    with (
        tc.tile_pool(name=f"sbA{rep}", bufs=1) as sbA,
        tc.tile_pool(name=f"wp{rep}", bufs=3) as wpool,
        tc.tile_pool(name=f"pp{rep}", bufs=2, space="PSUM") as ppool,
        tc.tile_pool(name=f"qk{rep}", bufs=2) as qkpool,
        tc.tile_pool(name=f"gp{rep}", bufs=2) as gpool,
        tc.tile_pool(name=f"sp{rep}", bufs=2, space="PSUM") as spool,
        tc.tile_pool(name=f"up{rep}", bufs=2, space="PSUM") as upool,
        tc.tile_pool(name=f"ep{rep}", bufs=NEB) as epool,
        tc.tile_pool(name=f"fin{rep}", bufs=2) as fpool,
        tc.tile_pool(name=f"dr{rep}", bufs=1, space="DRAM") as dpool,
    ):
        # ---- resident inputs
        ball_sb = sbA.tile([128, 3 * G], F32, tag="ball_sb")
        nc.sync.dma_start(out=ball_sb[:], in_=ball[:])
        xhi = sbA.tile([128, KT * TPC], FP8, tag="xhi")      # [e_lo, (k, t)]
        xlo = sbA.tile([128, KT * TPC], FP8, tag="xlo")
        xhv = xhi[:].rearrange("p (k t) -> p k t", k=KT)
        xlv = xlo[:].rearrange("p (k t) -> p k t", k=KT)
        vaug = sbA.tile([128, (G + 1) * TPC], BF16, tag="vaug")  # [dv,(g,t)]+ones
        nc.vector.memset(vaug[:, G * TPC :], 1.0)
        q_dr = dpool.tile([D, G, TPC], BF16, tag="q_dr")
        k_dr = dpool.tile([D, G, TPC], BF16, tag="k_dr")

        # ---- weight prefetch queue: wt DMAs are hoisted out of project() so
        # they can be issued ahead on the SP queue (wpool bufs throttle depth)
        wt_order = ([(wk, g) for g in range(G)]
                    + [(wq, g) for g in range(G)]
                    + [(wv, g) for g in range(G)])
        wt_ready = []

        def prefetch_wt():
            if not wt_order:
                return
            wext, g = wt_order.pop(0)
            wt = wpool.tile([128, KT * 2 * 128], FP8, tag="wt")
            nc.sync.dma_start(out=wt[:], in_=wext[g])
            wt_ready.append(wt)

        # ---- projections: 3-product fp8 DoubleRow into one psum per group.
        # t0/t1 select a token range (Q runs as two half-token passes so the
        # first half of stage C can start ~30us earlier; Q weights stream
        # twice for that). When `chunked`, yields every VCHUNK matmuls so the
        # caller can interleave score sub-batches at the matching runtime rate.
        def project(bias_base, kind, g, t0=0, t1=TPC, chunked=False):
            wt = wt_ready.pop(0)
            wv4 = wt[:].rearrange("p (k s d) -> p k s d", k=KT, s=2)
            ps = ppool.tile([128, TPC], F32, tag="ps")
            nt = t1 - t0
            NI = 3 * (KT // 2)
            n = 0
            for wslot, xsrc in ((0, xhv), (1, xhv), (0, xlv)):
                for kk in range(0, KT, 2):
                    nc.tensor.matmul(
                        ps[:, :nt],
                        wv4[:, kk : kk + 2, wslot, :],
                        xsrc[:, kk : kk + 2, t0:t1],
                        start=(n == 0), stop=(n == NI - 1),
                        perf_mode=DR,
                    )
                    n += 1
                    if n == NI // 2:
                        prefetch_wt()
                    if chunked and n % VCHUNK == 0 and n < NI:
                        yield
            bias_ap = ball_sb[:, bias_base + g : bias_base + g + 1]
            if kind == "v":
                nc.vector.tensor_scalar(
                    vaug[:, g * TPC + t0 : g * TPC + t1], ps[:, :nt],
                    1.0 / WPS, bias_ap,
                    mybir.AluOpType.mult, mybir.AluOpType.add,
                )
            else:
                stg = qkpool.tile([128, TPC], BF16, tag="stg")
                nc.vector.tensor_scalar(
                    stg[:, :nt], ps[:, :nt], 1.0 / WPS, bias_ap,
                    mybir.AluOpType.mult, mybir.AluOpType.add,
                )
                dst = q_dr if kind == "q" else k_dr
                nc.sync.dma_start(out=dst[:, g, t0:t1], in_=stg[:, :nt])

        def run(gen):
            for _ in gen:
                pass

        # first weight tile + x halves; weights first so PE starts earliest
        prefetch_wt()
        nc.sync.dma_start(out=xhv[:, :16, :], in_=xhi_ext[:, :16, :])
        prefetch_wt()
        nc.sync.dma_start(out=xhv[:, 16:, :], in_=xhi_ext[:, 16:, :])
        nc.sync.dma_start(out=xlv[:, :16, :], in_=xlo_ext[:, :16, :])
        nc.sync.dma_start(out=xlv[:, 16:, :], in_=xlo_ext[:, 16:, :])

        vv = vaug[:].rearrange("p (n t) -> p t n", t=TPC)
        ebs = {}

        def u_block(b):
            """U matmuls + normalize + output DMA for block b."""
            eb = ebs.pop(b)
            t0 = b * CB
            up = upool.tile([128, CB * (G + 1)], F32, tag="up")
            for i in range(CB):
                nc.tensor.matmul(
                    up[:, i * (G + 1) : (i + 1) * (G + 1)],
                    eb[:, i * D : (i + 1) * D], vv[:, t0 + i, :],
                    start=True, stop=True,
                )
            uv = up[:].rearrange("d (t n) -> d t n", n=G + 1)
            rtd = fpool.tile([128, CB], F32, tag="rtd")
            nc.vector.reciprocal(rtd[:], uv[:, :, G])
            at = fpool.tile([128, CB * G], F32, tag="at")
            nc.vector.tensor_tensor(
                at[:].rearrange("d (t g) -> d t g", g=G),
                uv[:, :, 0:G],
                rtd[:].unsqueeze(2).broadcast_to([128, CB, G]),
                op=mybir.AluOpType.mult,
            )
            nc.sync.dma_start(out=out_ext[:, t0 : t0 + CB, :], in_=at[:])

        HB = NCB // 2
        HT = TPC // 2

        def sc_subbatches(blocks):
            for b in blocks:
                t0 = b * CB
                qg = gpool.tile([G, D * CB], BF16, tag="qg")
                kg = gpool.tile([G, D * CB], BF16, tag="kg")
                nc.sync.dma_start(
                    out=qg[:], in_=q_dr[:, :, t0 : t0 + CB].transpose([1, 0, 2])
                )
                nc.sync.dma_start(
                    out=kg[:], in_=k_dr[:, :, t0 : t0 + CB].transpose([1, 0, 2])
                )
                eb = epool.tile([128, CB * D], BF16, tag="eb")
                ebs[b] = eb
                qv = qg[:].rearrange("g (d t) -> g d t", t=CB)
                kv = kg[:].rearrange("g (d t) -> g d t", t=CB)
                for sbi in range(NSB):
                    sp = spool.tile([128, SBT * D], F32, tag="sp")   # 2 banks
                    for i in range(SBT):
                        tl = sbi * SBT + i
                        nc.tensor.matmul(
                            sp[:, i * D : (i + 1) * D],
                            kv[:, :, tl], qv[:, :, tl],
                            start=True, stop=True,
                        )
                    nc.scalar.activation(
                        eb[:, sbi * SBT * D : (sbi + 1) * SBT * D], sp[:], AF.Exp
                    )
                    yield

        def run(gen):
            for _ in gen:
                pass

        def interleave(proj_gens, sb_gen):
            """Alternate [VCHUNK proj matmuls][one score sub-batch]."""
            for gen in proj_gens:
                alive = True
                while alive:
                    try:
                        next(gen)
                    except StopIteration:
                        alive = False
                    next(sb_gen, None)
            for _ in sb_gen:
                pass

        # phase A: K then Q, all groups, dense
        for g in range(G):
            run(project(G, "k", g))
        for g in range(G):
            run(project(0, "q", g))
        # phase B: V projections interleaved 1:1 with the first DEFER blocks'
        # score sub-batches (48 V chunks : 48 sub-batches)
        interleave([project(2 * G, "v", g, chunked=True) for g in range(G)],
                   sc_subbatches(range(DEFER)))
        # phase C: rolling blocks' scores interleaved with all U blocks
        sc_c = sc_subbatches(range(DEFER, NCB))
        n_sb_c = (NCB - DEFER) * NSB
        u_done = 0
        for i in range(n_sb_c):
            next(sc_c, None)
            want = (i + 1) * NCB // n_sb_c
            while u_done < min(want, NCB):
                u_block(u_done)
                u_done += 1
        while u_done < NCB:
            u_block(u_done)
            u_done += 1


def build_program(reps=1):
    """Build the SPMD single-core program; same NEFF runs on all 8 cores."""
    nc = bass.Bass("TRN2", target_bir_lowering=False, debug=False,
                   num_devices=NCORES)
    params = {
        "xhi": nc.declare_dram_parameter("xhi", [128, KT, TPC], FP8, isOutput=False),
        "xlo": nc.declare_dram_parameter("xlo", [128, KT, TPC], FP8, isOutput=False),
        "wq": nc.declare_dram_parameter("wq", [G, 128, KT, 2, 128], FP8, isOutput=False),
        "wk": nc.declare_dram_parameter("wk", [G, 128, KT, 2, 128], FP8, isOutput=False),
        "wv": nc.declare_dram_parameter("wv", [G, 128, KT, 2, 128], FP8, isOutput=False),
        "ball": nc.declare_dram_parameter("ball", [128, 3 * G], F32, isOutput=False),
        "out": nc.declare_dram_parameter("out", [D, TPC, G], F32, isOutput=True),
    }
    with _SplitDrainTileContext(nc) as tc:
        params["_tc"] = tc
        for rep in range(reps):
            _emit_body(nc, params, rep)
    del params["_tc"]
    return nc


def prepare_inputs(x, Wq, bq, Wk, bk, Wv, bv):
    """Host-side sharding + layout/precision transforms -> per-core in_maps."""
    x = np.asarray(x, np.float32)
    scale = np.float32(1.0 / np.sqrt(D))

    def wmat(W, do_sum):
        W = np.asarray(W, np.float32)
        if do_sum:
            W = W.reshape(E, D, G, SC).sum(axis=3) * scale
        W = W * np.float32(WPS)
        # [E, D, G] -> [E, (g, d)] -> [k, p, g, d]
        m = W.transpose(0, 2, 1).reshape(E, G * D)
        t4 = m.reshape(KT, 128, G, D)
        hi = t4.astype(F8).astype(np.float32)
        lo = (t4 - hi).astype(F8)
        hi = hi.astype(F8)
        # [g, p, k, slot, d]
        arr = np.stack(
            [hi.transpose(2, 1, 0, 3), lo.transpose(2, 1, 0, 3)], axis=3
        )
        return np.ascontiguousarray(arr)

    wq_h = wmat(Wq, True)
    wk_h = wmat(Wk, False)
    wv_h = wmat(Wv, False)
    bq_h = (np.asarray(bq, np.float32).reshape(D, G, SC).sum(axis=2) * scale)
    ball_h = np.ascontiguousarray(np.concatenate(
        [bq_h, np.asarray(bk, np.float32), np.asarray(bv, np.float32)], axis=1
    ))

    x_flat = x.reshape(T, E)
    in_maps = []
    for i in range(NCORES):
        xT = x_flat[i * TPC : (i + 1) * TPC].T          # [E, TPC]
        xw = xT.reshape(KT, 128, TPC).transpose(1, 0, 2)  # [p, k, t]
        xhi = xw.astype(F8).astype(np.float32)
        xlo = (xw - xhi).astype(F8)
        in_maps.append({
            "xhi": np.ascontiguousarray(xhi.astype(F8)),
            "xlo": np.ascontiguousarray(xlo),
            "wq": wq_h, "wk": wk_h, "wv": wv_h,
            "ball": ball_h,
        })
    return in_maps


def finalize_output(core_outs):
    """[D, TPC, G] per core -> full [B, S, E] f32 (transpose + head dup)."""
    parts = []
    for o in core_outs:
        o = np.asarray(o, np.float32).transpose(1, 0, 2)      # [TPC, D, G]
        o = np.broadcast_to(o[:, :, :, None], (TPC, D, G, SC))
        parts.append(o.reshape(TPC, E))
    return np.concatenate(parts, axis=0).reshape(B, S, E)


_CACHED = {}


def kernel(x, Wq, bq, Wk, bk, Wv, bv):
    from concourse.bass_utils import run_bass_kernel_spmd

    if "nc" not in _CACHED:
        _CACHED["nc"] = build_program(reps=1)
    nc = _CACHED["nc"]
    in_maps = prepare_inputs(x, Wq, bq, Wk, bk, Wv, bv)
    res = run_bass_kernel_spmd(nc, in_maps, list(range(NCORES)), trace=False)
    return finalize_output([res.results[i]["out"] for i in range(NCORES)])
